# revision 38
# baseline (speedup 1.0000x reference)
"""Trainium2 Bass kernel for nn_Attention_73254962200646.

Reference computation (per batch element b, all shapes hardcoded):
  qkv = conv3x3(x, W_qkv, pad=1)            x:[8,512,32,32], W_qkv:[1536,512,3,3]
  q,k,v -> [g=8 heads, n=1024, d=64]
  attn  = (q @ k^T) / (|q| |k| + eps)       cosine-similarity attention
  out   = attn @ v -> [512,32,32]
  out   = conv1x1(out, W_out); BatchNorm2d (batch stats); ReLU

Distribution: data-parallel over batch B=8 across the 8 NeuronCores (one
image per core). All compute is core-local in bf16 (fp32 PSUM accumulation);
the only collective is a 4KB AllReduce of the BatchNorm partial sums.

The emission order interleaves phases at ~2us granularity so the PE never
idles waiting on PSUM->SBUF copies (which would also drop the HAM clock
from 2.4 to 1.2 GHz): each block's epilogue (copies, norms, transposes)
is woven into the next block's conv matmuls, and each attention pair is
woven into the following conv block.
"""

import numpy as np
import ml_dtypes

import concourse.tile as tile
import concourse.mybir as mybir
from concourse import bacc, bass_utils

BF = ml_dtypes.bfloat16
SMOOTH = 1e-4
BN_EPS = 1e-5
NCORES = 8

_NC = None
LAST_RESULT = None


def _build():
    f32 = mybir.dt.float32
    bf = mybir.dt.bfloat16
    AF = mybir.ActivationFunctionType
    ALU = mybir.AluOpType

    nc = bacc.Bacc("TRN2", target_bir_lowering=False, debug=False,
                   num_devices=NCORES)
    xin = nc.dram_tensor("xpad", [4, 128, 34, 34], bf, kind="ExternalInput").ap()
    wqk = nc.dram_tensor("wqk", [12, 4, 128, 3, 3, 128], bf, kind="ExternalInput").ap()
    wo = nc.dram_tensor("wo", [4, 128, 512], bf, kind="ExternalInput").ap()
    gb = nc.dram_tensor("gb", [128, 8], f32, kind="ExternalInput").ap()
    ones2 = nc.dram_tensor("ones2", [128, 2], bf, kind="ExternalInput").ap()
    sel2 = nc.dram_tensor("sel2", [2, 128], f32, kind="ExternalInput").ap()
    ident = nc.dram_tensor("ident", [128, 128], bf, kind="ExternalInput").ap()
    out = nc.dram_tensor("out", [512, 1024], f32, kind="ExternalOutput").ap()

    with tile.TileContext(nc) as tc:
        with tc.tile_pool(name="sb", bufs=1) as sb, \
             tc.tile_pool(name="tp", bufs=2) as tp, \
             tc.tile_pool(name="ps", bufs=4, space="PSUM") as ps, \
             tc.tile_pool(name="dram", bufs=1, space="DRAM") as dram:

            xps = [sb.tile([128, 34, 34], bf, tag=f"xp{cb}", name=f"xp{cb}")
                   for cb in range(4)]
            identt = sb.tile([128, 128], bf, tag="identt")
            wot = sb.tile([128, 4, 512], bf, tag="wot")
            gbt = sb.tile([128, 8], f32, tag="gbt")
            ones2t = sb.tile([128, 2], bf, tag="ones2t")
            sel2t = sb.tile([2, 128], f32, tag="sel2t")
            qhat = sb.tile([128, 4, 1024], bf, tag="qhat")
            khat = sb.tile([128, 4, 1024], bf, tag="khat")
            vT = sb.tile([128, 8, 512], bf, tag="vT")
            att = sb.tile([128, 4, 1024], bf, tag="att")
            yt = sb.tile([128, 4, 1024], f32, tag="yt")
            part = sb.tile([128, 8], f32, tag="part")
            stats = sb.tile([128, 8], f32, tag="stats")
            epst = sb.tile([128, 1], f32, tag="epst")
            smt = sb.tile([2, 1], f32, tag="smt")

            # startup DMAs: sync queue is reserved for the weight stream
            # (the first conv chunk needs wqk[8,0] + xp[0] as early as possible)
            nc.scalar.dma_start(xps[0][:], xin[0])
            nc.gpsimd.dma_start(xps[1][:], xin[1])
            nc.scalar.dma_start(xps[2][:], xin[2])
            nc.gpsimd.dma_start(xps[3][:], xin[3])
            nc.gpsimd.dma_start(identt[:], ident)
            nc.gpsimd.dma_start(ones2t[:], ones2)
            nc.gpsimd.dma_start(sel2t[:], sel2)
            for cb in range(4):
                nc.gpsimd.dma_start(wot[:, cb], wo[cb])
            nc.gpsimd.dma_start(gbt[:], gb)
            nc.vector.memset(epst[:], BN_EPS)
            nc.vector.memset(smt[:], SMOOTH)

            def emit_warm_ar():
                # tiny warm-up AllReduce: pays the ncfw cold-entry cost while
                # the convs run, so the tail BN AllReduce enters a warm path
                warm_in = dram.tile([1, 8], f32, name="warm_in")
                warm_out = dram.tile([1, 8], f32, name="warm_out")
                warm_sb = sb.tile([1, 8], f32, tag="warm_sb")
                nc.vector.memset(warm_sb[:], 0.0)
                nc.gpsimd.dma_start(warm_in[:], warm_sb[:])
                nc.gpsimd.collective_compute(
                    "AllReduce", ALU.add,
                    ins=[warm_in[:].opt()], outs=[warm_out[:].opt()],
                    replica_groups=[list(range(NCORES))])

            def conv_gen(cob):
                """Yields (pq, raw) after DMA issue, then None per 9-MM chunk."""
                wqts = [tp.tile([128, 3, 3, 128], bf, tag=f"wq{cb}", bufs=3,
                                name=f"wqt{cob}_{cb}") for cb in range(4)]
                for cb in range(4):
                    nc.sync.dma_start(wqts[cb][:], wqk[cob, cb])
                pq = ps.tile([128, 1024], f32, tag="mmp", bufs=4,
                             name=f"pq{cob}")
                raw = tp.tile([128, 1024], bf, tag="raw", bufs=4,
                              name=f"raw{cob}")
                yield (pq, raw)
                for t in range(2):
                    k = 0
                    for cb in range(4):
                        for ky in range(3):
                            for kx in range(3):
                                nc.tensor.matmul(
                                    pq[:, 512 * t:512 * (t + 1)],
                                    wqts[cb][:, ky, kx, :],
                                    xps[cb][:, 16 * t + ky:16 * t + ky + 16,
                                            kx:kx + 32],
                                    start=(k == 0), stop=(k == 35))
                                k += 1
                        yield None

            def post_gen(cob, pq, raw):
                """Epilogue for a conv block: psum copy, then per-kind tail."""
                nc.scalar.copy(raw[:, 0:512], pq[:, 0:512])
                nc.vector.tensor_copy(out=raw[:, 512:1024], in_=pq[:, 512:1024])
                yield None
                if cob >= 8:   # v block: PE-transpose into vT
                    m = cob - 8
                    for c2 in range(2):
                        pt = ps.tile([128, 512], bf, tag="mmp", bufs=4,
                                     name=f"pt{cob}_{c2}")
                        for c in range(4):
                            j = 4 * c2 + c
                            nc.tensor.transpose(pt[:, 128 * c:128 * (c + 1)],
                                                raw[:, 128 * j:128 * (j + 1)],
                                                identt[:])
                        dstv = vT[:, 4 * c2:4 * (c2 + 1), 128 * m:128 * (m + 1)]
                        srcv = pt[:].rearrange("p (a b) -> p a b", a=4)
                        if c2 == 0:
                            nc.scalar.copy(dstv, srcv)
                        else:
                            nc.vector.tensor_copy(out=dstv, in_=srcv)
                        yield None
                else:          # q/k block: cosine norms + normalized copy
                    m = cob % 4
                    dst = qhat if cob < 4 else khat
                    nrm = tp.tile([2, 1024], f32, tag="nrm", bufs=2,
                                  name=f"nrm{cob}")
                    inv = tp.tile([2, 1024], f32, tag="inv", bufs=2,
                                  name=f"inv{cob}")
                    sq = tp.tile([128, 1024], bf, tag="sq", bufs=2,
                                 name=f"sq{cob}")
                    nc.scalar.square(sq[:, 0:512], raw[:, 0:512])
                    nc.vector.tensor_mul(sq[:, 512:1024], raw[:, 512:1024],
                                         raw[:, 512:1024])
                    yield None
                    for t in range(2):
                        pss = ps.tile([2, 512], f32, tag="mmp", bufs=4,
                                      name=f"pss{cob}_{t}")
                        nc.tensor.matmul(pss[:], ones2t[:],
                                         sq[:, 512 * t:512 * (t + 1)],
                                         start=True, stop=True)
                        nc.scalar.activation(out=nrm[:, 512 * t:512 * (t + 1)],
                                             in_=pss[:], func=AF.Sqrt,
                                             bias=smt[:], scale=1.0)
                        yield None
                    nc.vector.reciprocal_approx_fast(out=inv[:], in_=nrm[:])
                    yield None
                    for t in range(2):
                        pbc = ps.tile([128, 512], f32, tag="mmp", bufs=4,
                                      name=f"pbc{cob}_{t}")
                        nc.tensor.matmul(pbc[:], sel2t[:],
                                         inv[:, 512 * t:512 * (t + 1)],
                                         start=True, stop=True)
                        nc.vector.tensor_mul(dst[:, m, 512 * t:512 * (t + 1)],
                                             raw[:, 512 * t:512 * (t + 1)],
                                             pbc[:])
                        yield None

            def att_gen(m):
                """Attention pair (heads 2m, 2m+1): 2 chunks per j block."""
                po = ps.tile([128, 1024], f32, tag="mmp", bufs=4, name=f"po{m}")
                prev = None
                for j in range(8):
                    if prev is not None:
                        emit_outT(m, po, *prev)
                    pa0 = ps.tile([128, 1024], f32, tag="mmp", bufs=4,
                                  name=f"pa0_{m}_{j}")
                    pa1 = ps.tile([128, 1024], f32, tag="mmp", bufs=4,
                                  name=f"pa1_{m}_{j}")
                    for t in range(2):
                        nc.tensor.matmul(pa0[:, 512 * t:512 * (t + 1)],
                                         khat[0:64, m, 128 * j:128 * (j + 1)],
                                         qhat[0:64, m, 512 * t:512 * (t + 1)],
                                         start=True, stop=True)
                        nc.tensor.matmul(pa1[:, 512 * t:512 * (t + 1)],
                                         khat[64:128, m, 128 * j:128 * (j + 1)],
                                         qhat[64:128, m, 512 * t:512 * (t + 1)],
                                         start=True, stop=True)
                    yield None
                    a0 = tp.tile([128, 1024], bf, tag="attnT", bufs=6,
                                 name=f"a0_{m}_{j}")
                    a1 = tp.tile([128, 1024], bf, tag="attnT", bufs=6,
                                 name=f"a1_{m}_{j}")
                    nc.scalar.copy(a0[:], pa0[:])
                    nc.vector.tensor_copy(out=a1[:], in_=pa1[:])
                    prev = (j, a0, a1)
                    yield None
                emit_outT(m, po, *prev)
                if m % 2 == 0:
                    nc.scalar.copy(att[:, m, :], po[:])
                else:
                    nc.vector.tensor_copy(out=att[:, m, :], in_=po[:])
                yield None

            def emit_outT(m, po, j, a0, a1):
                for t in range(2):
                    nc.tensor.matmul(po[0:64, 512 * t:512 * (t + 1)],
                                     vT[:, j, 128 * m:128 * m + 64],
                                     a0[:, 512 * t:512 * (t + 1)],
                                     start=(j == 0), stop=(j == 7),
                                     tile_position=(0, 0))
                    nc.tensor.matmul(po[64:128, 512 * t:512 * (t + 1)],
                                     vT[:, j, 128 * m + 64:128 * (m + 1)],
                                     a1[:, 512 * t:512 * (t + 1)],
                                     start=(j == 0), stop=(j == 7),
                                     tile_position=(0, 64))

            def conv1x1_gen():
                for c4 in range(4):
                    py = ps.tile([128, 1024], f32, tag="mmp", bufs=4,
                                 name=f"py{c4}")
                    for t in range(2):
                        for cb in range(4):
                            nc.tensor.matmul(py[:, 512 * t:512 * (t + 1)],
                                             wot[:, cb, 128 * c4:128 * (c4 + 1)],
                                             att[:, cb, 512 * t:512 * (t + 1)],
                                             start=(cb == 0), stop=(cb == 3))
                    yield None
                    nc.vector.tensor_scalar(
                        out=yt[:, c4, :], in0=py[:],
                        scalar1=1.0, scalar2=None,
                        op0=ALU.mult, op1=ALU.add,
                        accum_out=part[:, c4:c4 + 1])
                    bscr = tp.tile([128, 1024], bf, tag="bscr", bufs=2,
                                   name=f"bscr{c4}")
                    nc.scalar.activation(out=bscr[:], in_=py[:], func=AF.Square,
                                         accum_out=part[:, 4 + c4:5 + c4])
                    yield None

            def drain(g):
                if g is not None:
                    for _ in g:
                        pass

            def interleave(main, filler, lead=1):
                """Drain `main`; after each of its chunks past `lead`,
                emit one chunk of `filler`."""
                i = 0
                for _ in main:
                    i += 1
                    if filler is not None and i > lead:
                        next(filler, None)
                drain(filler)

            def block_gen(cob):
                g = conv_gen(cob)
                pq, raw = next(g)
                yield from g
                yield from post_gen(cob, pq, raw)

            # ---- emission plan ----
            # v and q blocks plus k4/k5 pipeline each epilogue into the next
            # block's conv; attention pairs 0/1 weave with k6/k7; pairs 2/3
            # weave with each other; conv1x1 reads all att blocks (written by
            # pair 3's last copy) so it follows plainly.
            pending_post = None
            for ib, cob in enumerate([8, 9, 10, 11, 0, 1, 2, 3, 4, 5]):
                g = conv_gen(cob)
                pq, raw = next(g)
                interleave(g, pending_post, lead=1)
                pending_post = post_gen(cob, pq, raw)
                if ib == 0:
                    emit_warm_ar()
            drain(pending_post)   # k5 norms

            interleave(att_gen(0), block_gen(6), lead=0)
            interleave(att_gen(1), block_gen(7), lead=0)
            interleave(att_gen(2), att_gen(3), lead=0)
            drain(conv1x1_gen())

            # ---- BatchNorm: AllReduce 4KB of partial sums, then apply ----
            cin_d = dram.tile([128, 8], f32)
            cout_d = dram.tile([128, 8], f32)
            nc.gpsimd.dma_start(cin_d[:], part[:])
            nc.gpsimd.collective_compute(
                "AllReduce", ALU.add,
                ins=[cin_d[:].opt()], outs=[cout_d[:].opt()],
                replica_groups=[list(range(NCORES))])
            nc.sync.dma_start(stats[:], cout_d[:])

            var = sb.tile([128, 4], f32, tag="var")
            stdt = sb.tile([128, 4], f32, tag="stdt")
            rstd = sb.tile([128, 4], f32, tag="rstd")
            scl = sb.tile([128, 4], f32, tag="scl")
            sht = sb.tile([128, 4], f32, tag="sht")
            msq = sb.tile([128, 4], f32, tag="msq")
            tmp = sb.tile([128, 4], f32, tag="tmp")
            NINV = 1.0 / 8192.0
            nc.vector.tensor_scalar_mul(stats[:], stats[:], NINV)
            mean = stats[:, 0:4]
            ex2 = stats[:, 4:8]
            nc.vector.tensor_mul(msq[:], mean[:], mean[:])
            nc.vector.tensor_sub(var[:], ex2[:], msq[:])
            nc.scalar.activation(out=stdt[:], in_=var[:], func=AF.Sqrt,
                                 bias=epst[:], scale=1.0)
            nc.vector.reciprocal_approx_fast(out=rstd[:], in_=stdt[:])
            nc.vector.tensor_mul(scl[:], gbt[:, 0:4], rstd[:])
            nc.vector.tensor_mul(tmp[:], mean[:], scl[:])
            nc.vector.tensor_sub(sht[:], gbt[:, 4:8], tmp[:])
            out_q = [nc.sync, nc.gpsimd, nc.sync, nc.gpsimd]
            for c4 in range(4):
                nc.scalar.activation(out=yt[:, c4, :], in_=yt[:, c4, :],
                                     func=AF.Relu,
                                     scale=scl[:, c4:c4 + 1],
                                     bias=sht[:, c4:c4 + 1])
                out_q[c4].dma_start(out[128 * c4:128 * (c4 + 1), :],
                                    yt[:, c4, :])

    nc.compile()
    return nc


def _prep_inputs(x, W_qkv, W_out, gamma, beta):
    x = np.asarray(x, np.float32)
    W_qkv = np.asarray(W_qkv, np.float32)
    W_out = np.asarray(W_out, np.float32)
    gamma = np.asarray(gamma, np.float32)
    beta = np.asarray(beta, np.float32)

    xs = x.reshape(8, 4, 128, 32, 32)
    xpad = np.zeros((8, 4, 128, 34, 34), np.float32)
    xpad[:, :, :, 1:33, 1:33] = xs
    xpad = xpad.astype(BF)

    wqk = np.ascontiguousarray(
        W_qkv.reshape(12, 128, 4, 128, 3, 3)
        .transpose(0, 2, 3, 4, 5, 1).astype(BF))
    wo = np.ascontiguousarray(
        W_out[:, :, 0, 0].T.reshape(4, 128, 512).astype(BF))
    gb = np.ascontiguousarray(np.concatenate(
        [gamma.reshape(4, 128).T, beta.reshape(4, 128).T], axis=1)
        .astype(np.float32))
    p = np.arange(128)
    ones2 = np.ascontiguousarray(
        np.stack([p < 64, p >= 64], axis=1).astype(BF))
    sel2 = np.ascontiguousarray(
        np.stack([p < 64, p >= 64], axis=0).astype(np.float32))
    identv = np.eye(128, dtype=BF)

    common = {"wqk": wqk, "wo": wo, "gb": gb,
              "ones2": ones2, "sel2": sel2, "ident": identv}
    return [{"xpad": np.ascontiguousarray(xpad[b]), **common}
            for b in range(8)]


def kernel(x, W_qkv, W_out, gamma, beta):
    global _NC, LAST_RESULT
    if _NC is None:
        _NC = _build()
    in_maps = _prep_inputs(x, W_qkv, W_out, gamma, beta)
    res = bass_utils.run_bass_kernel_spmd(
        _NC, in_maps, core_ids=list(range(NCORES)))
    LAST_RESULT = res
    outs = [res.results[b]["out"].reshape(512, 32, 32) for b in range(8)]
    return np.stack(outs).astype(np.float32)
